# revision 10
# baseline (speedup 1.0000x reference)
"""Multi-head causal attention (B=4, T=2048, D=512, H=8) on 8 TRN2 NeuronCores.

Sharding: core c handles batch b = c//2 and head-group hg = c%2 (4 heads,
256 output dims).  No collectives needed — 8 fully independent problems.

Per-core algorithm (matmul inputs bf16, O^T accumulation f32 in PSUM):
  - host passes x^T (D,T) and W^T slices (D, 256) in bf16 + a [128,128]
    triangular causal mask
  - Q^T,K^T projections:  qT[dh2,T] = W2h @ xT, two heads stacked per tile
  - V projection into augmented-V tiles [k-tile 128, 65] (ones column
    appended -> the O^T matmul also produces the softmax denominator row)
  - flash-style: S^T[k,q] = K^T.T @ Q^T per (k-tile, q-block) written to
    bf16 PSUM, exp via one ACT activation per 4 k-tiles (scale=1/8 folded
    in; no max subtraction: |scores| < ~4).  Causal: diagonal k-tiles only
    compute q >= k-tile start, triangle-mask multiply on boundary blocks.
  - O^T accumulated in PSUM over k-tiles, then PE-transpose + divide by
    denominator -> natural [T,256] -> DMA out

Scheduling (program order == Tile priority):  K/Q projections for head
group 0 first, V-projection units woven between the first head's
score/exp batches (ACT starts ~12us in), group-1 projections emitted
after head 1 as PE filler during the ACT-bound phase.  qb runs DESCENDING
so attention opens with its PE-densest stretch — the PE clock (HAM) never
throttles down; re-warming from cold needs a fully-busy 3.4us window that
sparse-qb units can't provide.
"""

import numpy as np
import ml_dtypes

T = 2048
D = 512
HG = 4  # heads per core
DH = 64
OUTW = HG * DH  # 256
QB = 512  # q block (columns of S^T tiles)
NQB = T // QB  # 4
NKT = T // 128  # 16 k-tiles
N_CORES = 8

_CACHE = {}


def _build_nc():
    import concourse.bacc as bacc
    import concourse.tile as tile
    import concourse.mybir as mybir
    from concourse.masks import make_identity
    from contextlib import ExitStack

    fp32 = mybir.dt.float32
    bf16 = mybir.dt.bfloat16
    EXP = mybir.ActivationFunctionType.Exp

    nc = bacc.Bacc(None, target_bir_lowering=False)

    xt_d = nc.declare_dram_parameter("xt", [D, T], bf16, isOutput=False)
    wqt_d = nc.declare_dram_parameter("wqt", [D, OUTW], bf16, isOutput=False)
    wkt_d = nc.declare_dram_parameter("wkt", [D, OUTW], bf16, isOutput=False)
    wvt_d = nc.declare_dram_parameter("wvt", [D, OUTW], bf16, isOutput=False)
    cmask_d = nc.declare_dram_parameter("cmask", [128, 128], bf16, isOutput=False)
    out_d = nc.declare_dram_parameter("out", [T, OUTW], fp32, isOutput=True)

    with tile.TileContext(nc) as tc, ExitStack() as ctx:
        const = ctx.enter_context(tc.tile_pool(name="const", bufs=1))
        ps_s = ctx.enter_context(tc.tile_pool(name="ps_s", bufs=2, space="PSUM"))
        pt_pool = ctx.enter_context(tc.tile_pool(name="pt", bufs=4))
        osb_pool = ctx.enter_context(tc.tile_pool(name="osb", bufs=2))
        rec_pool = ctx.enter_context(tc.tile_pool(name="rec", bufs=4))

        # ---- input loads: weights first (first matmuls need them), x on the
        # scalar HWDGE queue for issue parallelism with sync
        def load4(dram, name, width, eng):
            ts = []
            for c in range(4):
                t = const.tile([128, width], bf16, tag=f"{name}{c}", name=f"{name}{c}")
                eng.dma_start(out=t[:], in_=dram[c * 128:(c + 1) * 128, :])
                ts.append(t)
            return ts

        wkT = load4(wkt_d, "wkT", OUTW, nc.sync)
        wqT = load4(wqt_d, "wqT", OUTW, nc.sync)
        xT = load4(xt_d, "xT", T, nc.scalar)
        wvT = load4(wvt_d, "wvT", OUTW, nc.sync)

        mask_sb = const.tile([128, 128], bf16, name="mask_sb")
        nc.sync.dma_start(out=mask_sb[:], in_=cmask_d[:])

        ident = const.tile([128, 128], fp32, name="ident")
        make_identity(nc, ident[:])

        # ---- persistent SBUF tensors ----
        qT = [const.tile([128, T], bf16, tag=f"qT{g}", name=f"qT{g}") for g in range(2)]
        kT = [const.tile([128, T], bf16, tag=f"kT{g}", name=f"kT{g}") for g in range(2)]
        vaug = const.tile([128, NKT, HG, 65], bf16, name="vaug")
        nc.vector.memset(vaug[:, :, :, 64:65], 1.0)
        out_sb = const.tile([128, NQB * 4, OUTW], fp32, tag="out_sb", name="out_sb")

        def proj_qk(dst, wt, g, qb4):
            ps = ps_s.tile([128, QB], fp32, tag="ps", name="ps")
            for c in range(4):
                nc.tensor.matmul(
                    ps[:],
                    wt[c][:, g * 128:(g + 1) * 128],
                    xT[c][:, qb4 * QB:(qb4 + 1) * QB],
                    start=(c == 0),
                    stop=(c == 3),
                )
            nc.vector.tensor_copy(dst[g][:, qb4 * QB:(qb4 + 1) * QB], ps[:])

        def proj_v(tt):
            ps = ps_s.tile([128, OUTW], fp32, tag="ps", name="ps")
            for c in range(4):
                nc.tensor.matmul(
                    ps[:],
                    xT[c][:, tt * 128:(tt + 1) * 128],
                    wvT[c][:, 0:OUTW],
                    start=(c == 0),
                    stop=(c == 3),
                )
            nc.vector.tensor_copy(
                vaug[:, tt, :, 0:64],
                ps[:].rearrange("p (h d) -> p h d", h=HG),
            )

        def attn_unit(qb, h, filler=None):
            """One (head, q-block) attention unit.  filler(i) is called
            between score-batch i and its exp to weave in other PE work."""
            g, po = h // 2, 64 * (h % 2)
            ot = ps_s.tile([128, QB], fp32, tag="ot", name="ot")
            last_kt = qb * 4 + 3

            def score_mm(st_ap, kt, q0, width):
                nc.tensor.matmul(
                    st_ap,
                    kT[g][po:po + 64, kt * 128:(kt + 1) * 128],
                    qT[g][po:po + 64, qb * QB + q0: qb * QB + q0 + width],
                    start=True,
                    stop=True,
                )

            def ot_mm(kt, pt_ap, q0, width):
                nc.tensor.matmul(
                    ot[0:65, q0:q0 + width],
                    vaug[:, kt, h, :],
                    pt_ap,
                    start=(kt == 0),
                    stop=(kt == last_kt),
                )

            nb = 0
            # off-diagonal k-tiles: 2 per PSUM tile -> one exp each
            for kt0 in range(0, qb * 4, 2):
                st = ps_s.tile([128, 2 * QB], fp32, tag="st", name="st")
                score_mm(st[:, 0:QB], kt0, 0, QB)
                score_mm(st[:, QB:2 * QB], kt0 + 1, 0, QB)
                if filler:
                    filler(nb)
                nb += 1
                pt = pt_pool.tile([128, 2 * QB], bf16, tag="pt", name="pt")
                nc.scalar.activation(pt[:], st[:], func=EXP, scale=0.125)
                ot_mm(kt0, pt[:, 0:QB], 0, QB)
                ot_mm(kt0 + 1, pt[:, QB:2 * QB], 0, QB)

            # diagonal k-tiles j=0..3, restricted to q >= j*128: widths
            # 512/384/256/128 packed pairwise into two PSUM tiles
            for ja, jb in ((0, 1), (2, 3)):
                wa, wb = QB - ja * 128, QB - jb * 128
                st = ps_s.tile([128, 2 * QB], fp32, tag="st", name="st")
                score_mm(st[:, 0:wa], qb * 4 + ja, ja * 128, wa)
                score_mm(st[:, wa:wa + wb], qb * 4 + jb, jb * 128, wb)
                if filler:
                    filler(nb)
                nb += 1
                pt = pt_pool.tile([128, 2 * QB], bf16, tag="pt", name="pt")
                nc.scalar.activation(
                    pt[:, 0:wa + wb], st[:, 0:wa + wb], func=EXP, scale=0.125
                )
                nc.vector.tensor_mul(pt[:, 0:128], pt[:, 0:128], mask_sb[:])
                nc.vector.tensor_mul(
                    pt[:, wa:wa + 128], pt[:, wa:wa + 128], mask_sb[:]
                )
                ot_mm(qb * 4 + ja, pt[:, 0:wa], ja * 128, wa)
                ot_mm(qb * 4 + jb, pt[:, wa:wa + wb], jb * 128, wb)

            # normalize + transpose to natural layout
            osb = osb_pool.tile([65, QB], fp32, tag="osb", name="osb")
            nc.vector.tensor_copy(osb[:], ot[0:65, :])
            for j4 in range(4):
                tp = ps_s.tile([128, 128], fp32, tag="ps", name="tp")
                nc.tensor.transpose(
                    tp[:, 0:65],
                    osb[:, j4 * 128:(j4 + 1) * 128],
                    ident[0:65, 0:65],
                )
                rec = rec_pool.tile([128, 1], fp32, tag="rec", name="rec")
                nc.vector.reciprocal(rec[:], tp[:, 64:65])
                nc.vector.tensor_scalar_mul(
                    out_sb[:, qb * 4 + j4, h * 64:(h + 1) * 64],
                    tp[:, 0:64],
                    rec[:],
                )

        def stream_out(qb):
            for j4 in range(4):
                tt = qb * 4 + j4
                nc.sync.dma_start(
                    out=out_d[tt * 128:(tt + 1) * 128, :], in_=out_sb[:, tt, :]
                )

        # ---- schedule ----
        for qb4 in range(4):
            proj_qk(kT, wkT, 0, qb4)
        for qb4 in (3, 2, 1, 0):  # attention consumes qT in descending qb
            proj_qk(qT, wqT, 0, qb4)

        # head 0, qb=3: V projections woven between score batches (2 V units
        # per batch; ot batch i needs vaug k-tiles 2i, 2i+1 = V units 2i, 2i+1)
        attn_unit(3, 0, filler=lambda i: [proj_v(tt) for tt in (2 * i, 2 * i + 1)])
        attn_unit(3, 1)
        # group-1 projections: PE filler during the ACT-bound phase
        for qb4 in range(4):
            proj_qk(kT, wkT, 1, qb4)
        for qb4 in (3, 2, 1, 0):
            proj_qk(qT, wqT, 1, qb4)
        attn_unit(3, 2)
        attn_unit(3, 3)
        stream_out(3)
        for qb in (2, 1, 0):
            for h in range(HG):
                attn_unit(qb, h)
            stream_out(qb)

    nc.finalize()
    return nc


def _get_nc():
    if "nc" not in _CACHE:
        _CACHE["nc"] = _build_nc()
    return _CACHE["nc"]


def _make_cmask():
    # triangle: mask[p, f] = 1.0 iff p <= f
    p = np.arange(128)[:, None]
    f = np.arange(128)[None, :]
    return (p <= f).astype(ml_dtypes.bfloat16)


def _make_in_maps(x, Wq, Wk, Wv):
    bf = ml_dtypes.bfloat16
    cmask = _make_cmask()
    in_maps = []
    for c in range(N_CORES):
        b, hg = c // 2, c % 2
        r0 = hg * OUTW
        in_maps.append({
            "xt": np.ascontiguousarray(x[b].T).astype(bf),
            "wqt": np.ascontiguousarray(Wq[r0:r0 + OUTW].T).astype(bf),
            "wkt": np.ascontiguousarray(Wk[r0:r0 + OUTW].T).astype(bf),
            "wvt": np.ascontiguousarray(Wv[r0:r0 + OUTW].T).astype(bf),
            "cmask": cmask,
        })
    return in_maps


def kernel(x, Wq, Wk, Wv):
    from concourse.bass_utils import run_bass_kernel_spmd

    nc = _get_nc()
    in_maps = _make_in_maps(x, Wq, Wk, Wv)
    res = run_bass_kernel_spmd(nc, in_maps, core_ids=list(range(N_CORES)))

    B = x.shape[0]
    out = np.empty((B, T, D), dtype=np.float32)
    for c in range(N_CORES):
        b, hg = c // 2, c % 2
        out[b, :, hg * OUTW:(hg + 1) * OUTW] = res.results[c]["out"]
    return out
